# revision 20
# baseline (speedup 1.0000x reference)
import sys

sys.path.insert(0, "/opt/trn_rl_repo")

import numpy as np

B, S, A = 65536, 376, 17
H1, H2, K = 400, 300, 15
NCORES = 8
NPC = B // NCORES  # samples per core
P = 128

UW_N = 2 * K * A  # 510 = [u | w] head width
MLB_N = 2 * A + K + 1  # 50 = [mu | lv | b | pad]; fp32r matmul needs even free sizes
LOG_2PI = float(np.log(2.0 * np.pi))


def _split_matmul_waits(nc):
    """This walrus build allows only 1 embedded sync wait per instruction
    (setupSyncWait asserts for Matmult/Activation/Drain/...). Move excess
    waits onto NoOps inserted just before, on the same engine queue
    (queue order makes the waits execute first, so semantics are identical)."""
    from concourse import mybir

    n_split = 0
    for func in nc.m.functions:
        for block in func.blocks:
            out = []
            for inst in block.instructions:
                si = inst.sync_info
                if si is not None and si.on_wait and len(si.on_wait) > 1:
                    waits = list(si.on_wait)
                    for wi, w in enumerate(waits[:-1]):
                        nop = mybir.InstNoOp(
                            name=f"{inst.name}-wsplit{wi}",
                            engine=inst.engine,
                            sync_info=mybir.SyncInfo(on_wait=[w], on_update=[]),
                            bass_nofuse=True,
                        )
                        nc.register_instruction(nop)
                        out.append(nop)
                    si.on_wait = waits[-1:]
                    n_split += 1
                out.append(inst)
            block.instructions = out
    return n_split


def _build_program(npc, C):
    """Emit the per-core Bass program. npc: samples per core, C: tiles/cohort."""
    import concourse.bass as bass
    import concourse.tile as tile
    from concourse import mybir
    from concourse.masks import make_identity

    NT = npc // P
    assert NT % C == 0
    ngroups = NT // 4 if NT >= 4 else 1
    tiles_per_group = min(4, NT)
    f32 = mybir.dt.float32
    f32r = mybir.dt.float32r
    AF = mybir.ActivationFunctionType
    OP = mybir.AluOpType

    def r(ap):
        return ap.bitcast(f32r)

    nc = bass.Bass()

    x_d = nc.dram_tensor("x", [npc, S], f32, kind="ExternalInput")
    eps_d = nc.dram_tensor("eps", [npc, A], f32, kind="ExternalInput")
    W1_d = nc.dram_tensor("W1", [S, H1], f32, kind="ExternalInput")
    b1_d = nc.dram_tensor("b1", [H1], f32, kind="ExternalInput")
    W2_d = nc.dram_tensor("W2", [H1, H2], f32, kind="ExternalInput")
    b2_d = nc.dram_tensor("b2", [H2], f32, kind="ExternalInput")
    Wuw_d = nc.dram_tensor("Wuw", [H2, UW_N], f32, kind="ExternalInput")
    buw_d = nc.dram_tensor("brow_uw", [1, UW_N], f32, kind="ExternalInput")
    Wmlb_d = nc.dram_tensor("Wmlb", [H2, MLB_N], f32, kind="ExternalInput")
    bmlb_d = nc.dram_tensor("brow_mlb", [1, MLB_N], f32, kind="ExternalInput")

    ones_d = nc.dram_tensor("ones_row", [1, P], f32, kind="ExternalInput")
    act_d = nc.dram_tensor("action", [npc, A], f32, kind="ExternalOutput")
    prob_d = nc.dram_tensor("prob", [npc], f32, kind="ExternalOutput")
    lprob_d = nc.dram_tensor("logprob", [npc], f32, kind="ExternalOutput")

    KC1 = [(0, 128), (128, 256), (256, S)]  # mm1 K chunks (features of x)
    MC1 = [(i * 100, (i + 1) * 100) for i in range(4)]  # H1 chunks
    KC2 = MC1  # mm2 contracts over H1, chunked as h1 tiles
    MC2 = [(i * 100, (i + 1) * 100) for i in range(3)]  # H2 chunks
    KCH = MC2  # heads contract over H2, chunked as h2 tiles

    with tile.TileContext(nc) as tc, bass.ExitStack() as ctx:
        singles = ctx.enter_context(tc.tile_pool(name="singles", bufs=1))
        xin = ctx.enter_context(tc.tile_pool(name="xin", bufs=3))
        epsin = ctx.enter_context(tc.tile_pool(name="epsin", bufs=3))
        xTp = ctx.enter_context(tc.tile_pool(name="xT", bufs=6))
        h1p = ctx.enter_context(tc.tile_pool(name="h1", bufs=8))
        h2p = ctx.enter_context(tc.tile_pool(name="h2", bufs=6))
        tmp = ctx.enter_context(tc.tile_pool(name="tmp", bufs=6))
        coh = ctx.enter_context(tc.tile_pool(name="coh", bufs=2))
        ps_tp = ctx.enter_context(tc.tile_pool(name="ps_tp", bufs=2, space="PSUM"))
        ps_mm = ctx.enter_context(tc.tile_pool(name="ps_mm", bufs=2, space="PSUM"))
        ps_uw = ctx.enter_context(tc.tile_pool(name="ps_uw", bufs=2, space="PSUM"))
        ps_mlb = ctx.enter_context(tc.tile_pool(name="ps_mlb", bufs=2, space="PSUM"))

        # ---- weights / constants (loaded once) ----
        ident = singles.tile([P, P], f32)
        make_identity(nc, ident)
        ones_t = singles.tile([1, P], f32r)
        nc.sync.dma_start(out=ones_t, in_=ones_d[:].bitcast(f32r))

        W1_t, b1_t = {}, {}
        for mi, (m0, m1) in enumerate(MC1):
            for ki, (k0, k1) in enumerate(KC1):
                t = singles.tile([k1 - k0, 100], f32r, name=f"W1_{ki}_{mi}")
                nc.sync.dma_start(out=t, in_=W1_d[k0:k1, m0:m1].bitcast(f32r))
                W1_t[ki, mi] = t
            bt = singles.tile([100, 1], f32, name=f"b1_{mi}")
            nc.sync.dma_start(out=bt, in_=b1_d[m0:m1].unsqueeze(-1))
            b1_t[mi] = bt
        W2_t, b2_t = {}, {}
        for mi, (m0, m1) in enumerate(MC2):
            for ki, (k0, k1) in enumerate(KC2):
                t = singles.tile([100, 100], f32r, name=f"W2_{ki}_{mi}")
                nc.sync.dma_start(out=t, in_=W2_d[k0:k1, m0:m1].bitcast(f32r))
                W2_t[ki, mi] = t
            bt = singles.tile([100, 1], f32, name=f"b2_{mi}")
            nc.sync.dma_start(out=bt, in_=b2_d[m0:m1].unsqueeze(-1))
            b2_t[mi] = bt
        Wuw_t, Wmlb_t = {}, {}
        for ki, (k0, k1) in enumerate(KCH):
            t = singles.tile([100, UW_N], f32r, name=f"Wuw_{ki}")
            nc.sync.dma_start(out=t, in_=Wuw_d[k0:k1, :].bitcast(f32r))
            Wuw_t[ki] = t
            t2 = singles.tile([100, MLB_N], f32r, name=f"Wmlb_{ki}")
            nc.sync.dma_start(out=t2, in_=Wmlb_d[k0:k1, :].bitcast(f32r))
            Wmlb_t[ki] = t2
        buw_t = singles.tile([1, UW_N], f32r)
        nc.sync.dma_start(out=buw_t, in_=buw_d[:].bitcast(f32r))
        bmlb_t = singles.tile([1, MLB_N], f32r)
        nc.sync.dma_start(out=bmlb_t, in_=bmlb_d[:].bitcast(f32r))

        cohort = {}

        def new_cohort():
            d = {}
            for nm, shp in (
                ("u", [P, K, C, A]),   # becomes u_hat in-place
                ("w", [P, K, C, A]),
                ("t4", [P, K, C, A]),  # scratch (wu / ww / w*coef)
                ("b", [P, K, C]),
                ("T", [P, K, C]),      # tanh(wzb) per step
                ("M", [P, K, C]),      # m_uw
                ("uw", [P, K, C]),
                ("wns", [P, K, C]),
                ("lc", [P, K, C]),     # log1p(exp(-uw))
                ("coef", [P, K, C]),
                ("z", [P, C, A]),
                ("lvs", [P, C]),
                ("eps2", [P, C]),
                ("wzb", [P, C]),
                ("ldj", [P, C]),
                ("lp", [P, C]),
                ("pr", [P, C]),
            ):
                d[nm] = coh.tile(shp, f32, name="coh_" + nm)
            return d

        def flow_and_output(d, ci):
            u, w, t4 = d["u"], d["w"], d["t4"]
            # uw = sum_a w*u ; wns = sum_a w*w
            nc.vector.tensor_tensor(out=t4, in0=w, in1=u, op=OP.mult)
            nc.vector.tensor_reduce(out=d["uw"], in_=t4, axis=mybir.AxisListType.X, op=OP.add)
            nc.scalar.square(out=t4, in_=w)
            nc.vector.tensor_reduce(out=d["wns"], in_=t4, axis=mybir.AxisListType.X, op=OP.add)
            # lc = log(1 + exp(-uw)); m_uw = uw + lc - 1; coef = (lc-1)/wns
            nc.scalar.activation(out=d["lc"], in_=d["uw"], func=AF.Exp, scale=-1.0)
            nc.scalar.activation(out=d["lc"], in_=d["lc"], func=AF.Ln, bias=1.0)
            nc.vector.scalar_tensor_tensor(
                out=d["M"], in0=d["lc"], scalar=-1.0, in1=d["uw"], op0=OP.add, op1=OP.add
            )
            nc.vector.reciprocal(out=d["coef"], in_=d["wns"])
            nc.vector.scalar_tensor_tensor(
                out=d["coef"], in0=d["lc"], scalar=-1.0, in1=d["coef"], op0=OP.add, op1=OP.mult
            )
            # u_hat = u + coef * w  (in place into u)
            coef_b = d["coef"][:].unsqueeze(-1).broadcast_to([P, K, C, A])
            nc.vector.tensor_tensor(out=t4, in0=w, in1=coef_b, op=OP.mult)
            nc.vector.tensor_tensor(out=u, in0=u, in1=t4, op=OP.add)
            # sequential flow
            z = d["z"]
            for k in range(K):
                wk = w[:, k]
                wz = t4[:, 0]  # [P, C, A] scratch
                nc.vector.tensor_tensor(out=wz, in0=wk, in1=z, op=OP.mult)
                nc.vector.tensor_reduce(out=d["wzb"], in_=wz, axis=mybir.AxisListType.X, op=OP.add)
                nc.vector.tensor_tensor(out=d["wzb"], in0=d["wzb"], in1=d["b"][:, k], op=OP.add)
                nc.scalar.activation(out=d["T"][:, k], in_=d["wzb"], func=AF.Tanh)
                t_b = d["T"][:, k].unsqueeze(-1).broadcast_to([P, C, A])
                nc.vector.tensor_tensor(out=wz, in0=u[:, k], in1=t_b, op=OP.mult)
                nc.vector.tensor_tensor(out=z, in0=z, in1=wz, op=OP.add)
            # ldj = sum_k log(1 + m_uw*(1-t^2))
            T, Mm = d["T"], d["M"]
            t2 = d["lc"]  # reuse
            nc.vector.tensor_tensor(out=t2, in0=T, in1=T, op=OP.mult)
            nc.scalar.activation(out=t2, in_=t2, func=AF.Copy, bias=1.0, scale=-1.0)
            nc.vector.tensor_tensor(out=t2, in0=t2, in1=Mm, op=OP.mult)
            nc.scalar.activation(out=t2, in_=t2, func=AF.Ln, bias=1.0)
            t2_t = t2[:].rearrange("p k c -> p c k")
            nc.vector.tensor_reduce(out=d["ldj"], in_=t2_t, axis=mybir.AxisListType.X, op=OP.add)
            # lp = -0.5*eps2 - lvs - 8.5*ln(2pi) - ldj ; pr = exp(lp)
            nc.vector.scalar_tensor_tensor(
                out=d["lp"], in0=d["eps2"], scalar=-0.5, in1=d["lvs"], op0=OP.mult, op1=OP.subtract
            )
            nc.vector.scalar_tensor_tensor(
                out=d["lp"], in0=d["lp"], scalar=-(A / 2.0) * LOG_2PI, in1=d["ldj"],
                op0=OP.add, op1=OP.subtract,
            )
            nc.scalar.activation(out=d["pr"], in_=d["lp"], func=AF.Exp)
            # outputs
            s0 = ci * C * P
            for jj in range(C):
                jt0 = s0 + jj * P
                nc.sync.dma_start(out=act_d[jt0 : jt0 + P, :], in_=z[:, jj, :])
                nc.sync.dma_start(out=prob_d[jt0 : jt0 + P].unsqueeze(-1), in_=d["pr"][:, jj : jj + 1])
                nc.sync.dma_start(out=lprob_d[jt0 : jt0 + P].unsqueeze(-1), in_=d["lp"][:, jj : jj + 1])

        for g in range(ngroups):
            x_t = []
            for j in range(tiles_per_group):
                t = xin.tile([P, S], f32, name=f"x{j}")
                r0 = (g * tiles_per_group + j) * P
                nc.sync.dma_start(out=t, in_=x_d[r0 : r0 + P, :])
                x_t.append(t)
            NW = tiles_per_group * P  # moving width this group
            # transpose x -> xT chunks [Kc, NW]
            xT = []
            for ki, (k0, k1) in enumerate(KC1):
                kc = k1 - k0
                tp = ps_tp.tile([P, NW], f32)
                for j in range(tiles_per_group):
                    nc.tensor.transpose(
                        out=tp[0:kc, j * P : (j + 1) * P],
                        in_=x_t[j][:, k0:k1],
                        identity=ident,
                    )
                st = xTp.tile([P, NW], f32r)
                nc.scalar.copy(out=st[0:kc, :], in_=tp[0:kc, :])
                xT.append(st)
            # mm1 -> h1 tiles [100, NW], relu+bias fused in eviction
            h1 = []
            for mi in range(4):
                ps = ps_mm.tile([100, NW], f32)
                for ki, (k0, k1) in enumerate(KC1):
                    kc = k1 - k0
                    nc.tensor.matmul(
                        out=ps, lhsT=r(W1_t[ki, mi]), rhs=r(xT[ki][0:kc, :]),
                        start=(ki == 0), stop=(ki == len(KC1) - 1),
                    )
                ht = h1p.tile([100, NW], f32r)
                nc.scalar.activation(out=ht, in_=ps, func=AF.Relu, bias=b1_t[mi])
                h1.append(ht)
            # mm2 -> h2 tiles [100, NW]
            h2 = []
            for mi in range(3):
                ps = ps_mm.tile([100, NW], f32)
                for ki in range(4):
                    nc.tensor.matmul(
                        out=ps, lhsT=r(W2_t[ki, mi]), rhs=r(h1[ki]),
                        start=(ki == 0), stop=(ki == 3),
                    )
                ht = h2p.tile([100, NW], f32r)
                nc.scalar.activation(out=ht, in_=ps, func=AF.Relu, bias=b2_t[mi])
                h2.append(ht)
            # heads, sample-major: out[j] = [128 samples, 510/49]
            for j in range(tiles_per_group):
                jt = g * tiles_per_group + j
                ci, jj = jt // C, jt % C
                if jj == 0:
                    cohort[ci] = new_cohort()
                d = cohort[ci]
                js = slice(j * P, (j + 1) * P)
                ps1 = ps_uw.tile([P, UW_N], f32)
                for ki in range(3):
                    nc.tensor.matmul(
                        out=ps1, lhsT=r(h2[ki][:, js]), rhs=r(Wuw_t[ki]),
                        start=(ki == 0), stop=False,
                    )
                nc.tensor.matmul(out=ps1, lhsT=r(ones_t), rhs=r(buw_t), start=False, stop=True)
                ps2 = ps_mlb.tile([P, MLB_N], f32)
                for ki in range(3):
                    nc.tensor.matmul(
                        out=ps2, lhsT=r(h2[ki][:, js]), rhs=r(Wmlb_t[ki]),
                        start=(ki == 0), stop=False,
                    )
                nc.tensor.matmul(out=ps2, lhsT=r(ones_t), rhs=r(bmlb_t), start=False, stop=True)
                # evictions
                nc.scalar.copy(out=d["u"][:, :, jj, :], in_=ps1[:, 0 : K * A].rearrange("p (k a) -> p k a", k=K))
                nc.scalar.copy(out=d["w"][:, :, jj, :], in_=ps1[:, K * A : UW_N].rearrange("p (k a) -> p k a", k=K))
                mlv = tmp.tile([P, 2 * A], f32)
                nc.scalar.activation(out=mlv, in_=ps2[:, 0 : 2 * A], func=AF.Tanh)
                nc.scalar.copy(out=d["b"][:, :, jj], in_=ps2[:, 2 * A : 2 * A + K])
                std = tmp.tile([P, A], f32)
                nc.scalar.activation(out=std, in_=mlv[:, A : 2 * A], func=AF.Exp)
                et = epsin.tile([P, A], f32)
                r0 = jt * P
                nc.sync.dma_start(out=et, in_=eps_d[r0 : r0 + P, :])
                # z = mu + std*eps
                se = tmp.tile([P, A], f32)
                nc.vector.tensor_tensor(out=se, in0=std, in1=et, op=OP.mult)
                nc.vector.tensor_tensor(out=d["z"][:, jj, :], in0=se, in1=mlv[:, 0:A], op=OP.add)
                # lvs, eps2
                nc.vector.tensor_reduce(
                    out=d["lvs"][:, jj : jj + 1], in_=mlv[:, A : 2 * A],
                    axis=mybir.AxisListType.X, op=OP.add,
                )
                sq = tmp.tile([P, A], f32)
                nc.scalar.activation(
                    out=sq, in_=et, func=AF.Square, accum_out=d["eps2"][:, jj : jj + 1]
                )
                if jj == C - 1:
                    flow_and_output(cohort.pop(ci), ci)

    _split_matmul_waits(nc)
    return nc


_NC_CACHE = {}


def _get_program(npc, C):
    key = (npc, C)
    if key not in _NC_CACHE:
        _NC_CACHE[key] = _build_program(npc, C)
    return _NC_CACHE[key]


def _prep_weights(i):
    Wuw = np.concatenate([i["Wu"], i["Ww"]], axis=1).astype(np.float32)
    brow_uw = np.concatenate([i["bu"], i["bw"]])[None, :].astype(np.float32)
    pad = np.zeros((H2, 1), np.float32)
    Wmlb = np.concatenate([i["Wmu"], i["Wlv"], i["Wb"], pad], axis=1).astype(np.float32)
    brow_mlb = np.concatenate([i["bmu"], i["blv"], i["bb"], [0.0]])[None, :].astype(np.float32)
    return {
        "W1": np.ascontiguousarray(i["W1"], np.float32),
        "b1": np.ascontiguousarray(i["b1"], np.float32),
        "W2": np.ascontiguousarray(i["W2"], np.float32),
        "b2": np.ascontiguousarray(i["b2"], np.float32),
        "Wuw": Wuw, "brow_uw": brow_uw, "Wmlb": Wmlb, "brow_mlb": brow_mlb,
        "ones_row": np.ones((1, P), np.float32),
    }


def kernel(**inputs):
    from concourse.bass_utils import run_bass_kernel_spmd

    nc = _get_program(NPC, 16)
    wts = _prep_weights(inputs)
    x = np.asarray(inputs["x"], np.float32)
    eps = np.asarray(inputs["eps"], np.float32)
    in_maps = []
    for c in range(NCORES):
        m = dict(wts)
        m["x"] = np.ascontiguousarray(x[c * NPC : (c + 1) * NPC])
        m["eps"] = np.ascontiguousarray(eps[c * NPC : (c + 1) * NPC])
        in_maps.append(m)
    res = run_bass_kernel_spmd(nc, in_maps, list(range(NCORES))).results
    action = np.concatenate([r["action"] for r in res], axis=0)
    prob = np.concatenate([r["prob"] for r in res], axis=0)
    logprob = np.concatenate([r["logprob"] for r in res], axis=0)
    return action, prob, logprob
